# revision 7
# baseline (speedup 1.0000x reference)
"""CRF log-partition (forward algorithm) on Trainium2, 8 NeuronCores.

Math
----
reference:  part_0[b,j] = pot[b,0,j] + trans[START,j]
            part_t[b,j] = pot[b,t,j] + LSE_i(part_{t-1}[b,i] + trans[i,j])   (masked update)
            out = sum_b LSE_i(part_S[b,i] + trans[i,STOP])

We run the scan in the exp domain.  With q_t = exp(part_t - c_t) (c a per-batch
log offset) and E = exp(trans), U_t = exp(pot[:,t,:]):

            q_t[j,b] = U_t[j,b] * sum_i E[i,j] q_{t-1}[i,b]

i.e. per step: one PE matmul with stationary lhsT (contraction over i on the
partition axis) followed by one DVE elementwise multiply.  State layout is
[tag(52 partitions), batch(free)] so the matmul output layout equals its input
layout and no transpose is ever needed.  E and q are bf16 (single-pass PE,
cheap LDWEIGHTS); PSUM accumulation and the elementwise path stay fp32, so the
only quantization is the 2^-9 relative rounding of the state per step, which
random-walks to ~1e-6 relative error on the final log-partition.

The stationary is E' = [E | ones] ([52, 53]): row 52 of every step's PSUM
output is sum_i q[i,b] for free, which the periodic renorm uses.

Masking: a masked step must freeze q.  Host prep bakes U_t=0 on masked (b,t)
(pot := -1e30) and ships V_t[j,b] = 1-mask so the device computes
q = z*U + q_prev*V (two extra DVE ops, only emitted for steps where any batch
is masked).

Stability: q drifts by ~2^10/step, so every RENORM steps we rescale each batch
column by r[b] = 1/sum_i q[i,b].  The scale application is folded off the
critical chain into the step multiply: W_{t+2} = U_{t+2} * broadcast(r) is
built while steps t..t+2 run, so the chain never sees the renorm (only the PE
rank-1 broadcast matmul + DVE reciprocal + W build run, all off-chain).
Every r is recorded; at the end c[b] = -sum_k ln r_k[b] restores the exact log
offset (bookkeeping is exact, so reciprocal precision does not affect
accuracy).

Finish:     out_b = ln(sum_i q_S[i,b] * E[i,STOP]) + c[b]
on device, summed over b on the host during unshard.

Sharding: data-parallel over batch: core k handles batches 8k..8k+7 (the full
sequence), transitions replicated.  One SPMD program; per-core data only.
"""

import numpy as np

T2 = 52
B, S = 64, 512
START_IDX, STOP_IDX = T2 - 2, T2 - 1
NCORES = 8
BC = B // NCORES  # batches per core
RENORM = 5  # renorm trigger period (see stability analysis above)

_CACHE: dict = {}


def _build(seq_len, masked_steps, n_batch):
    """Build + compile the Bass program.

    masked_steps: sorted tuple of t values (1 <= t < seq_len) for which the
    freeze path is emitted.  Program structure depends only on this tuple.
    """
    from contextlib import ExitStack

    import concourse.bacc as bacc
    import concourse.mybir as mybir
    import concourse.tile as tile

    f32 = mybir.dt.float32
    bf16 = mybir.dt.bfloat16
    Exp = mybir.ActivationFunctionType.Exp
    Ln = mybir.ActivationFunctionType.Ln

    nm = len(masked_steps)
    mstep_ix = {t: i for i, t in enumerate(masked_steps)}

    # renorm triggers: scale read from step t's PSUM row 52, applied at t+2
    trig = [t for t in range(1, seq_len) if t % RENORM == RENORM - 1 and t + 2 <= seq_len - 1]
    apply_at = {t + 2: k for k, t in enumerate(trig)}
    trig_set = set(trig)
    nren = len(trig)

    nc = bacc.Bacc("TRN2", target_bir_lowering=False, debug=False, num_devices=NCORES)

    pot = nc.dram_tensor("pot", [T2, seq_len * n_batch], f32, kind="ExternalInput").ap()
    trans = nc.dram_tensor("trans", [T2, T2], f32, kind="ExternalInput").ap()
    transT = nc.dram_tensor("transT", [T2, T2], f32, kind="ExternalInput").ap()
    if nm:
        vm = nc.dram_tensor("vm", [T2, nm * n_batch], f32, kind="ExternalInput").ap()
    fout = nc.dram_tensor("fout", [1, n_batch], f32, kind="ExternalOutput").ap()

    with tile.TileContext(nc) as tc, ExitStack() as ctx, nc.allow_low_precision(
        reason="bf16 scan state: 2^-9 per-step rounding random-walks to ~1e-6 "
        "relative error on the final log-partition; renorm bookkeeping is exact"
    ):
        const = ctx.enter_context(tc.tile_pool(name="const", bufs=1))
        raw = ctx.enter_context(tc.tile_pool(name="raw", bufs=2))
        qp = ctx.enter_context(tc.tile_pool(name="qp", bufs=4))
        sm = ctx.enter_context(tc.tile_pool(name="sm", bufs=2))
        wp = ctx.enter_context(tc.tile_pool(name="wp", bufs=2))
        pz = ctx.enter_context(tc.tile_pool(name="pz", bufs=4, space="PSUM"))
        ps = ctx.enter_context(tc.tile_pool(name="ps", bufs=2, space="PSUM"))
        prb = ctx.enter_context(tc.tile_pool(name="prb", bufs=2, space="PSUM"))

        # dummy first ACT so walrus' table load overlaps the input DMAs
        dmy = const.tile([1, 1], f32, tag="dmy")
        nc.vector.memset(dmy[:], 0.0)
        nc.scalar.activation(dmy[:], dmy[:], Exp)

        # --- constants ---
        t_sb = const.tile([T2, T2], f32, tag="t_sb")
        nc.sync.dma_start(t_sb[:], trans[:, :])
        # stationary E' = [exp(trans) | 0-pad | ones], bf16.  The ones column
        # sits at output partition 64 (PSUM partition offsets must be 0/32/64/96
        # for the engines), so row 64 of every step's PSUM is sum_i q[i,b].
        SROW = 64
        E_sb = const.tile([T2, SROW + 1], bf16, tag="E_sb")
        nc.scalar.activation(E_sb[:, 0:T2], t_sb[:], Exp)
        nc.vector.memset(E_sb[:, T2:SROW], 0.0)
        nc.vector.memset(E_sb[:, SROW : SROW + 1], 1.0)

        tTs = const.tile([T2, 1], f32, tag="tTs")
        nc.sync.dma_start(tTs[:], transT[:, START_IDX : START_IDX + 1])
        vstart = const.tile([T2, 1], f32, tag="vstart")
        nc.scalar.activation(vstart[:], tTs[:], Exp)

        ones_row = const.tile([1, T2], bf16, tag="ones_row")
        nc.vector.memset(ones_row[:], 1.0)

        if nm:
            vm_sb = const.tile([T2, nm * n_batch], f32, tag="vm_sb")
            nc.sync.dma_start(vm_sb[:], vm[:, :])

        # --- potentials: DMA + exp in chunks ---
        U_sb = const.tile([T2, seq_len * n_batch], f32, tag="U_sb")
        chunk = 512
        total = seq_len * n_batch
        for a in range(0, total, chunk):
            b_ = min(a + chunk, total)
            rawt = raw.tile([T2, b_ - a], f32, tag="rawc")
            nc.sync.dma_start(rawt[:], pot[:, a:b_])
            nc.scalar.activation(U_sb[:, a:b_], rawt[:], Exp)

        # r history for exact renorm bookkeeping
        if nren:
            r_hist = const.tile([1, nren * n_batch], bf16, tag="r_hist")

        # --- init: q0 = U_0 * exp(trans[START,:]) ---
        q = qp.tile([T2, n_batch], bf16, tag="q")
        nc.vector.tensor_scalar_mul(q[:], U_sb[:, 0:n_batch], vstart[:])

        # --- scan ---
        w_pending = {}  # t -> (W tile, V' tile or None)
        for t in range(1, seq_len):
            sl = slice(t * n_batch, (t + 1) * n_batch)
            psum_z = pz.tile([SROW + 1, n_batch], f32, tag="pz")
            nc.tensor.matmul(psum_z[:], E_sb[:], q[:], start=True, stop=True)

            u_ap = U_sb[:, sl]
            v_ap = None
            if t in mstep_ix:
                mi = mstep_ix[t]
                v_ap = vm_sb[:, mi * n_batch : (mi + 1) * n_batch]
            if t in w_pending:
                w_t, vp_t = w_pending.pop(t)
                u_ap = w_t[:]
                if vp_t is not None:
                    v_ap = vp_t[:]

            if t in mstep_ix:
                qv = sm.tile([T2, n_batch], f32, tag="qv")
                nc.vector.tensor_mul(qv[:], q[:], v_ap)
                t1 = sm.tile([T2, n_batch], f32, tag="t1")
                nc.vector.tensor_mul(t1[:], psum_z[0:T2, :], u_ap)
                q_new = qp.tile([T2, n_batch], bf16, tag="q")
                nc.vector.tensor_add(q_new[:], t1[:], qv[:])
            else:
                q_new = qp.tile([T2, n_batch], bf16, tag="q")
                nc.vector.tensor_mul(q_new[:], psum_z[0:T2, :], u_ap)
            q = q_new

            if t in trig_set:
                k = trig.index(t)
                rsl = slice(k * n_batch, (k + 1) * n_batch)
                nc.vector.reciprocal(r_hist[:, rsl], psum_z[SROW : SROW + 1, :])
                rb = prb.tile([T2, n_batch], f32, tag="prb")
                nc.tensor.matmul(
                    rb[:], ones_row[:], r_hist[:, rsl], start=True, stop=True
                )
                ta = t + 2
                asl = slice(ta * n_batch, (ta + 1) * n_batch)
                w_t = wp.tile([T2, n_batch], f32, tag="wf")
                nc.vector.tensor_mul(w_t[:], U_sb[:, asl], rb[:])
                vp_t = None
                if ta in mstep_ix:
                    ma = mstep_ix[ta]
                    vp_t = wp.tile([T2, n_batch], f32, tag="vf")
                    nc.vector.tensor_mul(
                        vp_t[:], vm_sb[:, ma * n_batch : (ma + 1) * n_batch], rb[:]
                    )
                w_pending[ta] = (w_t, vp_t)

        # --- finish: f[b] = ln(sum_i q[i,b] E[i,STOP]) - sum_k ln r_k[b] ---
        psum_f = ps.tile([1, n_batch], f32, tag="ps")
        nc.tensor.matmul(
            psum_f[:], E_sb[:, STOP_IDX : STOP_IDX + 1], q[:], start=True, stop=True
        )
        lnf = sm.tile([1, n_batch], f32, tag="lnf")
        nc.scalar.activation(lnf[:], psum_f[:], Ln)
        if nren:
            lnr = const.tile([1, nren * n_batch], f32, tag="lnr")
            nc.scalar.activation(lnr[:], r_hist[:], Ln)
            c_t = sm.tile([1, n_batch], f32, tag="c_t")
            # view [1, b, k] with k innermost; sum over k, negated
            lnr_v = lnr.rearrange("p (k b) -> p b k", b=n_batch)
            nc.vector.reduce_sum(
                c_t[:].rearrange("p (b o) -> p b o", o=1), lnr_v,
                axis=mybir.AxisListType.X, negate=True,
            )
            f_sb = sm.tile([1, n_batch], f32, tag="f_sb")
            nc.vector.tensor_add(f_sb[:], lnf[:], c_t[:])
        else:
            f_sb = lnf
        nc.sync.dma_start(fout[:, :], f_sb[:])

    # Keep waits on MATMUL rather than hoisting them onto LDWEIGHTS: the
    # stationary (E') is a constant written once at startup, so its load can
    # prefetch during the wait for the DVE-produced rhs, taking ~127ns/step
    # off the serial chain.  PE executes in order, so the load still cannot
    # overtake the previous MATMUL.
    nc.move_matmul_waits_to_ldweights = lambda: None
    nc.compile()
    return nc


def _prep(potentials, transitions, mask, seq_len, n_batch, ncores):
    """Host-side shard + layout prep. Returns (masked_steps, in_maps)."""
    potentials = np.asarray(potentials, dtype=np.float32)
    transitions = np.asarray(transitions, dtype=np.float32)
    mask = np.asarray(mask).astype(bool)

    live = mask.copy()
    live[:, 0] = True  # t=0 init is unconditional in the reference
    masked_steps = tuple(int(t) for t in range(1, seq_len) if not live[:, t].all())

    potm = potentials.copy()
    potm[~live] = -1e30  # exp -> 0 on dead (b,t)

    trans_c = np.ascontiguousarray(transitions)
    transT_c = np.ascontiguousarray(transitions.T)

    in_maps = []
    for c in range(ncores):
        sl = slice(c * n_batch, (c + 1) * n_batch)
        pc = np.ascontiguousarray(
            np.transpose(potm[sl], (2, 1, 0)).reshape(T2, seq_len * n_batch)
        )
        m = {"pot": pc, "trans": trans_c, "transT": transT_c}
        if masked_steps:
            v = (~live[sl][:, list(masked_steps)]).astype(np.float32)  # [nb, nm]
            vt = np.ascontiguousarray(
                np.broadcast_to(v.T[None, :, :], (T2, len(masked_steps), n_batch))
                .reshape(T2, len(masked_steps) * n_batch)
            )
            m["vm"] = vt
        in_maps.append(m)
    return masked_steps, in_maps


def kernel(potentials, transitions, mask):
    from concourse.bass_utils import run_bass_kernel_spmd

    masked_steps, in_maps = _prep(potentials, transitions, mask, S, BC, NCORES)

    key = (S, masked_steps, BC)
    if key not in _CACHE:
        _CACHE[key] = _build(S, masked_steps, BC)
    nc = _CACHE[key]

    res = run_bass_kernel_spmd(nc, in_maps, core_ids=list(range(NCORES)))
    total = np.float32(0.0)
    for r in res.results:
        total += r["fout"].astype(np.float32).sum(dtype=np.float32)
    return np.array(total, dtype=np.float32)


# revision 13
# speedup vs baseline: 1.5757x; 1.5757x over previous
"""CRF log-partition (forward algorithm) on Trainium2, 8 NeuronCores.

Math
----
reference:  part_0[b,j] = pot[b,0,j] + trans[START,j]
            part_t[b,j] = pot[b,t,j] + LSE_i(part_{t-1}[b,i] + trans[i,j])   (masked update)
            out = sum_b LSE_i(part_S[b,i] + trans[i,STOP])

We run the scan in the exp domain.  With q_t = exp(part_t - c_t) (c a per-batch
log offset) and E = exp(trans), U_t = exp(pot[:,t,:]):

            q_t[j,b] = U_t[j,b] * sum_i E[i,j] q_{t-1}[i,b]

i.e. per step: one PE matmul with stationary lhsT (contraction over i on the
partition axis) followed by one DVE elementwise multiply.  State layout is
[tag(52 partitions), batch(free)] so the matmul output layout equals its input
layout and no transpose is ever needed.  E and q are bf16 (single-pass PE,
cheap LDWEIGHTS); PSUM accumulation and the elementwise path stay fp32, so the
only quantization is the 2^-9 relative rounding of the state per step, which
random-walks to ~1e-6 relative error on the final log-partition.

The stationary is E' = [E | ones] ([52, 53]): row 52 of every step's PSUM
output is sum_i q[i,b] for free, which the periodic renorm uses.

Masking: a masked step must freeze q.  Host prep bakes U_t=0 on masked (b,t)
(pot := -1e30) and ships V_t[j,b] = 1-mask so the device computes
q = z*U + q_prev*V (two extra DVE ops, only emitted for steps where any batch
is masked).

Stability: q drifts by ~2^10/step, so every RENORM steps we rescale each batch
column by r[b] = 1/sum_i q[i,b].  The scale application is folded off the
critical chain into the step multiply: W_{t+2} = U_{t+2} * broadcast(r) is
built while steps t..t+2 run, so the chain never sees the renorm (only the PE
rank-1 broadcast matmul + DVE reciprocal + W build run, all off-chain).
Every r is recorded; at the end c[b] = -sum_k ln r_k[b] restores the exact log
offset (bookkeeping is exact, so reciprocal precision does not affect
accuracy).

Finish:     out_b = ln(sum_i q_S[i,b] * E[i,STOP]) + c[b]
on device, summed over b on the host during unshard.

Sharding: data-parallel over batch: core k handles batches 8k..8k+7 (the full
sequence), transitions replicated.  One SPMD program; per-core data only.
"""

import numpy as np

T2 = 52
B, S = 64, 512
START_IDX, STOP_IDX = T2 - 2, T2 - 1
NCORES = 8
BC = B // NCORES  # batches per core
RENORM = 5  # renorm trigger period (see stability analysis above)

_CACHE: dict = {}


def _build(seq_len, masked_steps, n_batch):
    """Build + compile the Bass program.

    masked_steps: sorted tuple of t values (1 <= t < seq_len) for which the
    freeze path is emitted.  Program structure depends only on this tuple.
    """
    from contextlib import ExitStack

    import concourse.bacc as bacc
    import concourse.mybir as mybir
    import concourse.tile as tile

    f32 = mybir.dt.float32
    bf16 = mybir.dt.bfloat16
    Exp = mybir.ActivationFunctionType.Exp
    Ln = mybir.ActivationFunctionType.Ln

    nm = len(masked_steps)
    mstep_ix = {t: i for i, t in enumerate(masked_steps)}
    mset = set(masked_steps)

    # meet point: balance forward chain (steps 1..t_meet) against backward
    # (steps seq_len-1..t_meet+1); masked steps cost ~1.4x on the chain
    cost = [0.0] * seq_len
    for t in range(1, seq_len):
        cost[t] = 1.4 if t in mset else 1.0
    pref = np.cumsum(cost)
    total_c = pref[-1]
    t_meet = min(
        range(4, seq_len - 4),
        key=lambda t: max(pref[t], total_c - pref[t]),
    )
    nfwd_, nbwd_ = t_meet, seq_len - 1 - t_meet

    # renorm triggers: scale read from the trigger step's PSUM sum row,
    # applied (folded into W) two steps later.  Force a final trigger whose
    # application lands on the last step of each half so h and g are both
    # renormalized near the meet (their product must fit in fp32).
    ftrig = sorted(
        {t for t in range(1, t_meet) if t % RENORM == RENORM - 1 and t + 2 <= t_meet}
        | ({t_meet - 2} if t_meet >= 4 else set())
    )
    ftrig_tgt = {t: t + 2 for t in ftrig}
    ftrig_set = set(ftrig)
    nren_f = len(ftrig)
    btrig = sorted(
        {k for k in range(1, nbwd_) if k % RENORM == RENORM - 1 and k + 2 <= nbwd_}
        | ({nbwd_ - 2} if nbwd_ >= 4 else set())
    )
    btrig_set = set(btrig)
    nren_b = len(btrig)
    nren = nren_f + nren_b

    nc = bacc.Bacc("TRN2", target_bir_lowering=False, debug=False, num_devices=NCORES)

    pot = nc.dram_tensor("pot", [T2, seq_len * n_batch], f32, kind="ExternalInput").ap()
    trans = nc.dram_tensor("trans", [T2, T2], f32, kind="ExternalInput").ap()
    transT = nc.dram_tensor("transT", [T2, T2], f32, kind="ExternalInput").ap()
    if nm:
        vm = nc.dram_tensor("vm", [T2, nm * n_batch], f32, kind="ExternalInput").ap()
    fout = nc.dram_tensor("fout", [1, n_batch], f32, kind="ExternalOutput").ap()

    with tile.TileContext(nc) as tc, ExitStack() as ctx, nc.allow_low_precision(
        reason="bf16 scan state: 2^-9 per-step rounding random-walks to ~1e-6 "
        "relative error on the final log-partition; renorm bookkeeping is exact"
    ):
        const = ctx.enter_context(tc.tile_pool(name="const", bufs=1))
        raw = ctx.enter_context(tc.tile_pool(name="raw", bufs=2))
        qp = ctx.enter_context(tc.tile_pool(name="qp", bufs=4))
        sm = ctx.enter_context(tc.tile_pool(name="sm", bufs=2))
        wp = ctx.enter_context(tc.tile_pool(name="wp", bufs=2))
        pz = ctx.enter_context(tc.tile_pool(name="pz", bufs=3, space="PSUM"))
        pzb = ctx.enter_context(tc.tile_pool(name="pzb", bufs=3, space="PSUM"))
        ps = ctx.enter_context(tc.tile_pool(name="ps", bufs=1, space="PSUM"))
        prb = ctx.enter_context(tc.tile_pool(name="prb", bufs=1, space="PSUM"))

        # dummy first ACT so walrus' table load overlaps the input DMAs
        dmy = const.tile([1, 1], f32, tag="dmy")
        nc.vector.memset(dmy[:], 0.0)
        nc.scalar.activation(dmy[:], dmy[:], Exp)

        # --- constants ---
        t_sb = const.tile([T2, T2], f32, tag="t_sb")
        nc.sync.dma_start(t_sb[:], trans[:, :])
        # stationary E' = [exp(trans) | 0-pad | ones], bf16.  The ones column
        # sits at output partition 64 (PSUM partition offsets must be 0/32/64/96
        # for the engines), so row 64 of every step's PSUM is sum_i q[i,b].
        SROW = 64
        E_sb = const.tile([T2, SROW + 1], bf16, tag="E_sb")
        nc.scalar.activation(E_sb[:, 0:T2], t_sb[:], Exp)
        nc.vector.memset(E_sb[:, T2:SROW], 0.0)
        nc.vector.memset(E_sb[:, SROW : SROW + 1], 1.0)

        tTs = const.tile([T2, 1], f32, tag="tTs")
        nc.sync.dma_start(tTs[:], transT[:, START_IDX : START_IDX + 1])
        vstart = const.tile([T2, 1], f32, tag="vstart")
        nc.scalar.activation(vstart[:], tTs[:], Exp)

        ones_row = const.tile([1, T2], bf16, tag="ones_row")
        nc.vector.memset(ones_row[:], 1.0)

        if nm:
            vm_sb = const.tile([T2, nm * n_batch], f32, tag="vm_sb")
            nc.sync.dma_start(vm_sb[:], vm[:, :])

        # --- potentials: DMA + exp in chunks, alternating ends so both the
        # forward (low t) and backward (high t) scans can start early ---
        U_sb = const.tile([T2, seq_len * n_batch], f32, tag="U_sb")
        chunk = 512
        total = seq_len * n_batch
        starts = list(range(0, total, chunk))
        order = []
        lo, hi = 0, len(starts) - 1
        while lo <= hi:
            order.append(starts[lo])
            if hi != lo:
                order.append(starts[hi])
            lo, hi = lo + 1, hi - 1
        for a in order:
            b_ = min(a + chunk, total)
            rawt = raw.tile([T2, b_ - a], f32, tag="rawc")
            nc.sync.dma_start(rawt[:], pot[:, a:b_])
            nc.scalar.activation(U_sb[:, a:b_], rawt[:], Exp)

        # backward stationary ET' = [exp(transT) | 0-pad | ones]
        tT_sb = const.tile([T2, T2], f32, tag="tT_sb")
        nc.sync.dma_start(tT_sb[:], transT[:, :])
        ET_sb = const.tile([T2, SROW + 1], bf16, tag="ET_sb")
        nc.scalar.activation(ET_sb[:, 0:T2], tT_sb[:], Exp)
        nc.vector.memset(ET_sb[:, T2:SROW], 0.0)
        nc.vector.memset(ET_sb[:, SROW : SROW + 1], 1.0)

        # vstop[i] = exp(trans[i, STOP]) -- backward init vector
        vstop = const.tile([T2, 1], f32, tag="vstop")
        nc.scalar.activation(vstop[:], t_sb[:, STOP_IDX : STOP_IDX + 1], Exp)

        # r history for exact renorm bookkeeping (fwd slots then bwd slots)
        r_hist = const.tile([1, max(nren, 1) * n_batch], bf16, tag="r_hist")

        # --- init ---
        # forward: h0 = U_0 * exp(trans[START,:])
        q = qp.tile([T2, n_batch], bf16, tag="q")
        nc.vector.tensor_scalar_mul(q[:], U_sb[:, 0:n_batch], vstart[:])
        # backward: g = vstop broadcast over batch columns
        gi = const.tile([T2, n_batch], f32, tag="gi")
        nc.vector.memset(gi[:], 1.0)
        g = qp.tile([T2, n_batch], bf16, tag="g")
        nc.vector.tensor_scalar_mul(g[:], gi[:], vstop[:])
        g_is_psum = False

        # --- interleaved forward/backward scan, meeting at t_meet ---
        nfwd = t_meet            # forward steps t = 1..t_meet
        nbwd = seq_len - 1 - t_meet  # backward steps t = seq_len-1 .. t_meet+1
        w_pend_f = {}  # fwd t -> (W tile, V' tile or None)
        w_pend_b = {}  # bwd t -> (W tile, V' tile or None)

        def u_v_aps(t, pend):
            sl = slice(t * n_batch, (t + 1) * n_batch)
            u_ap = U_sb[:, sl]
            v_ap = None
            if t in mstep_ix:
                mi = mstep_ix[t]
                v_ap = vm_sb[:, mi * n_batch : (mi + 1) * n_batch]
            if t in pend:
                w_t, vp_t = pend.pop(t)
                u_ap = w_t[:]
                if vp_t is not None:
                    v_ap = vp_t[:]
            return u_ap, v_ap

        def emit_renorm(s_row, ta, pend, slot, fix_zero):
            rsl = slice(slot * n_batch, (slot + 1) * n_batch)
            if fix_zero:
                # backward sum is over U*g, exactly 0 for fully-dead columns:
                # add 1[sum==0] so those columns get r=1 (a no-op rescale)
                sfix = sm.tile([1, n_batch], f32, tag="sfix")
                nc.vector.tensor_scalar(
                    sfix[:], s_row, 0.0, None, mybir.AluOpType.is_equal
                )
                s2 = sm.tile([1, n_batch], f32, tag="s2")
                nc.vector.tensor_add(s2[:], sfix[:], s_row)
                nc.vector.reciprocal(r_hist[:, rsl], s2[:])
            else:
                nc.vector.reciprocal(r_hist[:, rsl], s_row)
            rb = prb.tile([T2, n_batch], f32, tag="prb")
            nc.tensor.matmul(rb[:], ones_row[:], r_hist[:, rsl], start=True, stop=True)
            asl = slice(ta * n_batch, (ta + 1) * n_batch)
            w_t = wp.tile([T2, n_batch], f32, tag="wf")
            nc.vector.tensor_mul(w_t[:], U_sb[:, asl], rb[:])
            vp_t = None
            if ta in mstep_ix:
                ma = mstep_ix[ta]
                vp_t = wp.tile([T2, n_batch], f32, tag="vf")
                nc.vector.tensor_mul(
                    vp_t[:], vm_sb[:, ma * n_batch : (ma + 1) * n_batch], rb[:]
                )
            pend[ta] = (w_t, vp_t)

        for k in range(1, max(nfwd, nbwd) + 1):
            if k <= nfwd:
                # ---- forward step t: h' = U_t*(E^T h) (+ V_t*h) ----
                t = k
                psum_z = pz.tile([SROW + 1, n_batch], f32, tag="pz")
                nc.tensor.matmul(psum_z[:], E_sb[:], q[:], start=True, stop=True)
                u_ap, v_ap = u_v_aps(t, w_pend_f)
                if v_ap is not None:
                    qv = sm.tile([T2, n_batch], f32, tag="qv")
                    nc.vector.tensor_mul(qv[:], q[:], v_ap)
                    t1 = sm.tile([T2, n_batch], f32, tag="t1")
                    nc.vector.tensor_mul(t1[:], psum_z[0:T2, :], u_ap)
                    q_new = qp.tile([T2, n_batch], bf16, tag="q")
                    nc.vector.tensor_add(q_new[:], t1[:], qv[:])
                else:
                    q_new = qp.tile([T2, n_batch], bf16, tag="q")
                    nc.vector.tensor_mul(q_new[:], psum_z[0:T2, :], u_ap)
                q = q_new
                if t in ftrig_set:
                    emit_renorm(
                        psum_z[SROW : SROW + 1, :], ftrig_tgt[t], w_pend_f,
                        ftrig.index(t), False,
                    )
            if k <= nbwd:
                # ---- backward step t: g' = E*(U_t*g) (+ V_t*g) ----
                t = seq_len - k
                u_ap, v_ap = u_v_aps(t, w_pend_b)
                g_ap = g[0:T2, :] if g_is_psum else g[:]
                m1 = qp.tile([T2, n_batch], bf16, tag="m1")
                nc.vector.tensor_mul(m1[:], g_ap, u_ap)
                psum_g = pzb.tile([SROW + 1, n_batch], f32, tag="pzb")
                nc.tensor.matmul(psum_g[:], ET_sb[:], m1[:], start=True, stop=True)
                if v_ap is not None:
                    gv = sm.tile([T2, n_batch], f32, tag="gv")
                    nc.vector.tensor_mul(gv[:], g_ap, v_ap)
                    g_new = qp.tile([T2, n_batch], bf16, tag="g")
                    nc.vector.tensor_add(g_new[:], psum_g[0:T2, :], gv[:])
                    g = g_new
                    g_is_psum = False
                else:
                    g = psum_g
                    g_is_psum = True
                if k in btrig_set:
                    emit_renorm(
                        psum_g[SROW : SROW + 1, :], seq_len - (k + 2), w_pend_b,
                        nren_f + btrig.index(k), True,
                    )

        # --- finish: f[b] = ln(sum_i h[i,b] g[i,b]) - sum_k ln r_k[b] ---
        g_ap = g[0:T2, :] if g_is_psum else g[:]
        hg = qp.tile([T2, n_batch], bf16, tag="hg")
        nc.vector.tensor_mul(hg[:], g_ap, q[:])
        psum_f = ps.tile([1, n_batch], f32, tag="ps")
        nc.tensor.matmul(
            psum_f[:], E_sb[:, SROW : SROW + 1], hg[:], start=True, stop=True
        )
        lnf = sm.tile([1, n_batch], f32, tag="lnf")
        nc.scalar.activation(lnf[:], psum_f[:], Ln)
        if nren:
            lnr = const.tile([1, nren * n_batch], f32, tag="lnr")
            nc.scalar.activation(lnr[:], r_hist[:], Ln)
            c_t = sm.tile([1, n_batch], f32, tag="c_t")
            # view [1, b, k] with k innermost; sum over k, negated
            lnr_v = lnr.rearrange("p (k b) -> p b k", b=n_batch)
            nc.vector.reduce_sum(
                c_t[:].rearrange("p (b o) -> p b o", o=1), lnr_v,
                axis=mybir.AxisListType.X, negate=True,
            )
            f_sb = sm.tile([1, n_batch], f32, tag="f_sb")
            nc.vector.tensor_add(f_sb[:], lnf[:], c_t[:])
        else:
            f_sb = lnf
        nc.sync.dma_start(fout[:, :], f_sb[:])

    nc.compile()
    return nc


def _prep(potentials, transitions, mask, seq_len, n_batch, ncores):
    """Host-side shard + layout prep. Returns (masked_steps, in_maps)."""
    potentials = np.asarray(potentials, dtype=np.float32)
    transitions = np.asarray(transitions, dtype=np.float32)
    mask = np.asarray(mask).astype(bool)

    live = mask.copy()
    live[:, 0] = True  # t=0 init is unconditional in the reference
    masked_steps = tuple(int(t) for t in range(1, seq_len) if not live[:, t].all())

    potm = potentials.copy()
    potm[~live] = -1e30  # exp -> 0 on dead (b,t)

    trans_c = np.ascontiguousarray(transitions)
    transT_c = np.ascontiguousarray(transitions.T)

    in_maps = []
    for c in range(ncores):
        sl = slice(c * n_batch, (c + 1) * n_batch)
        pc = np.ascontiguousarray(
            np.transpose(potm[sl], (2, 1, 0)).reshape(T2, seq_len * n_batch)
        )
        m = {"pot": pc, "trans": trans_c, "transT": transT_c}
        if masked_steps:
            v = (~live[sl][:, list(masked_steps)]).astype(np.float32)  # [nb, nm]
            vt = np.ascontiguousarray(
                np.broadcast_to(v.T[None, :, :], (T2, len(masked_steps), n_batch))
                .reshape(T2, len(masked_steps) * n_batch)
            )
            m["vm"] = vt
        in_maps.append(m)
    return masked_steps, in_maps


def kernel(potentials, transitions, mask):
    from concourse.bass_utils import run_bass_kernel_spmd

    masked_steps, in_maps = _prep(potentials, transitions, mask, S, BC, NCORES)

    key = (S, masked_steps, BC)
    if key not in _CACHE:
        _CACHE[key] = _build(S, masked_steps, BC)
    nc = _CACHE[key]

    res = run_bass_kernel_spmd(nc, in_maps, core_ids=list(range(NCORES)))
    total = np.float32(0.0)
    for r in res.results:
        total += r["fout"].astype(np.float32).sum(dtype=np.float32)
    return np.array(total, dtype=np.float32)
